# revision 7
# baseline (speedup 1.0000x reference)
"""FAPE loss Trainium2 kernel.

Math: for frames f (built from coord triples) and points n,
  d2[f,n] = ||Rp(p_n - po_f)||^2 + ||Rt(t_n - to_f)||^2 - 2 (p_n-po_f)^T M (t_n-to_f)
with M = Rp^T Rt.  Expanding, d2[f,n] = X[n] . Y[f] with 17 features:
  X = [A_n, 1, p (3), t (3), W (9)]   A_n = ||p_n||^2 + ||t_n||^2, W = outer(p_n, t_n)
  Y = [mask, B_f - 2c_f + off, 2(u-po), 2(v-to), -2M]  u = M to, v = M^T po,
      c_f = po.u, B_f = ||po||^2 + ||to||^2
Loss = mean(min(sqrt(d2 + eps), 10)) / 10.

The O(N) feature prep (X per point, Y per frame) is done host-side in numpy
and shipped pre-transposed in the exact matmul layouts, so the device does
only the O(F*N) part: 32 fp32r matmuls (K=17), ACT sqrt, DVE clamped
accumulation, and a scalar reduce.

Sharding: frames split across 8 cores (512/core; the last core's 2 pad
frames have all-zero Y rows).  Points replicated.

Device layout per core:
  xt [96, 1408] f32r: X^T in 11 windows of 128 cols (=128 points); window b,
      slot s in {0..2} holds feature k at partition 32s+k for point group
      g = 3b + s (points g*128 .. g*128+127); 33rd group slot zero.
  yt [96, 512] f32r: Y^T (features k on partitions) replicated at partition
      bases 0/32/64 so every lhsT slot finds a matching rhs.
  8 supertiles u of 4 matmuls g = 4u+h (window g//3, slot g%3) -> PSUM
      [128, 2048] f32 -> ACT sqrt(+eps) -> bf16 SBUF s.
      Finish on DVE in fast 16-bit mode: tmp = min(s, 10); acc += tmp
      (min also squashes any NaN from f32r noise on near-zero d2).
      Tail: row-reduce acc split ACT/DVE halves, DMA [128,1] out; host
      sums partitions.
"""
import sys

for _p in ("/opt/trn_rl_repo", "/root/.axon_site/_ro/trn_rl_repo"):
    if _p not in sys.path:
        sys.path.append(_p)

import numpy as np
from concourse import bass, bacc, mybir, tile
from concourse.bass_utils import run_bass_kernel_spmd

F32 = mybir.dt.float32
F32R = mybir.dt.float32r
BF16 = mybir.dt.bfloat16
AF = mybir.ActivationFunctionType
OP = mybir.AluOpType

N = 4096          # points
F = N - 2         # frames (4094)
NCORES = 8
FPC = 512         # frames per core (last core: 510 real + 2 zero-pad)
KF = 17           # contraction features
EPS = 1e-8
UNIT = 10.0
CLAMP = 10.0
DSQ_OFF = 1.0     # added to every real frame's d2 so f32r noise can't push
                  # it negative (sqrt(neg)=NaN); ~3.9e-4 relative loss bias
NWIN = 11         # X^T windows of 128 points, 3 feature-slots each
NST = 8           # supertiles of 2048 cols (4 matmuls each)


def build_nc():
    nc = bacc.Bacc(None)

    xt_d = nc.dram_tensor("xt", [96, 1408], F32R, kind="ExternalInput")
    yt_d = nc.dram_tensor("yt", [96, FPC], F32R, kind="ExternalInput")
    out_d = nc.dram_tensor("out", [128, 1], F32, kind="ExternalOutput")

    with tile.TileContext(nc) as tc:
        with (
            tc.tile_pool(name="inp", bufs=1) as inp,
            tc.tile_pool(name="sp", bufs=2) as sp,
            tc.tile_pool(name="accp", bufs=1) as accp,
            tc.tile_pool(name="psD", bufs=2, space="PSUM") as psD,
        ):
            xt_sb = inp.tile([96, 1408], F32R)
            yt_sb = inp.tile([96, FPC], F32R)
            # all input DMAs on the sync HWDGE queue: issuing from the
            # scalar queue delays the DMA semaphore behind ACT table loads
            nc.sync.dma_start(yt_sb[:], yt_d[:])
            nc.sync.dma_start(xt_sb[:, 0:256], xt_d[:, 0:256])
            nc.sync.dma_start(xt_sb[:, 256:1408], xt_d[:, 256:1408])

            epst = inp.tile([128, 1], F32)
            nc.vector.memset(epst[:], EPS)

            acc = accp.tile([128, 2048], BF16)

            for u in range(NST):
                ps = psD.tile([128, 2048], F32, tag="d2")
                for h in range(4):
                    g = 4 * u + h
                    b, sl = divmod(g, 3)
                    lhsT = xt_sb[32 * sl: 32 * sl + KF,
                                 b * 128: (b + 1) * 128]
                    rhs = yt_sb[32 * sl: 32 * sl + KF, :]
                    nc.tensor.matmul(
                        ps[:, h * FPC: (h + 1) * FPC], lhsT, rhs,
                        start=True, stop=True,
                    )
                s = sp.tile([128, 2048], BF16, tag="s")
                nc.scalar.activation(s[:], ps[:], AF.Sqrt, bias=epst[:])
                if u == 0:
                    nc.vector.tensor_scalar_min(acc[:], s[:], CLAMP)
                else:
                    tmp = sp.tile([128, 2048], BF16, tag="tmp")
                    nc.vector.tensor_scalar_min(tmp[:], s[:], CLAMP)
                    nc.vector.tensor_add(acc[:], acc[:], tmp[:])

            # tail: row sums split across ACT (left half) and DVE (right),
            # combined to [128, 1] and sent out; host sums partitions
            dumpA = sp.tile([128, 2048], BF16, tag="s")
            srowA = accp.tile([128, 1], F32)
            nc.scalar.activation(dumpA[:, 0:1024], acc[:, 0:1024], AF.Copy,
                                 accum_out=srowA[:])
            dumpB = sp.tile([128, 2048], BF16, tag="tmp")
            srowB = accp.tile([128, 1], F32)
            nc.vector.tensor_scalar(
                dumpB[:, 0:1024], acc[:, 1024:2048], 0.0, None, OP.add,
                OP.add, accum_out=srowB[:])
            nc.vector.tensor_add(srowA[:], srowA[:], srowB[:])
            nc.sync.dma_start(out_d[:], srowA[:])

    nc.finalize()
    return nc


_NC_CACHE = None


def _get_nc():
    global _NC_CACHE
    if _NC_CACHE is None:
        _NC_CACHE = build_nc()
    return _NC_CACHE


def _frames(c):
    o = c[1:-1]
    e1 = c[2:] - c[1:-1]
    e1 = e1 / (np.linalg.norm(e1, axis=1, keepdims=True) + EPS)
    e2 = c[:-2] - c[1:-1]
    e2 = e2 - (e2 * e1).sum(1, keepdims=True) * e1
    e2 = e2 / (np.linalg.norm(e2, axis=1, keepdims=True) + EPS)
    e3 = np.cross(e1, e2)
    R = np.stack([e1, e2, e3], 1)          # [F,3,3], rows are basis vecs
    return o, R


def make_in_maps(pred_coords, true_coords):
    pred = np.ascontiguousarray(pred_coords, dtype=np.float32)
    true = np.ascontiguousarray(true_coords, dtype=np.float32)

    # X features [N, 17]
    A = (pred * pred).sum(1) + (true * true).sum(1)
    W = (pred[:, :, None] * true[:, None, :]).reshape(N, 9)
    X = np.concatenate(
        [A[:, None], np.ones((N, 1), np.float32), pred, true, W],
        axis=1).astype(np.float32)

    # Y features [F, 17]
    po, Rp = _frames(pred)
    to, Rt = _frames(true)
    M = np.einsum('frc,frd->fcd', Rp, Rt)      # Rp^T Rt
    u = np.einsum('fcd,fd->fc', M, to)
    v = np.einsum('fcd,fc->fd', M, po)
    cf = (po * u).sum(1)
    B = (po * po).sum(1) + (to * to).sum(1)
    Y = np.concatenate(
        [np.ones((F, 1), np.float32), (B - 2 * cf + DSQ_OFF)[:, None],
         2 * (u - po), 2 * (v - to), (-2 * M).reshape(F, 9)],
        axis=1).astype(np.float32)

    # X^T layout [96, 1408]: xt[32s + k, b*128 + c] = X[(3b + s)*128 + c, k]
    # (33rd group slot unused/zero)
    xt = np.zeros((96, 1408), np.float32)
    Xp = np.zeros((NWIN * 3 * 128, KF), np.float32)
    Xp[:N] = X
    tmp = Xp.reshape(NWIN, 3, 128, KF)         # [b, s, c, k]
    xt.reshape(3, 32, NWIN, 128)[:, :KF] = tmp.transpose(1, 3, 0, 2)

    in_maps = []
    for i in range(NCORES):
        f0 = i * FPC
        nvalid = min(FPC, F - f0)
        Yc = np.zeros((FPC, KF), np.float32)
        Yc[:nvalid] = Y[f0: f0 + nvalid]
        yt = np.zeros((96, FPC), np.float32)
        yt.reshape(3, 32, FPC)[:, :KF] = Yc.T[None]
        in_maps.append({"xt": xt, "yt": yt})
    return in_maps


def kernel(pred_coords, true_coords):
    nc = _get_nc()
    in_maps = make_in_maps(pred_coords, true_coords)
    res = run_bass_kernel_spmd(nc, in_maps, list(range(NCORES)))
    total = sum(float(r["out"].sum()) for r in res.results)
    return np.float32(total / (F * N) / UNIT)
